# revision 2
# baseline (speedup 1.0000x reference)
import jax
import jax.numpy as jnp
import numpy as np

# nn_ClassifierDeformable: 6 deformable-conv layers (fixed offsets shared
# across batch) + 2-layer MLP head, data-parallel across the NeuronCores
# (batch 256 -> 32 per core, weights/offsets replicated).
#
# The deformable bilinear sampling has offsets shared across batch and
# channels, so each layer's gather+blend is a fixed linear map of the
# input spatial grid. We materialize that map on the host as a dense
# matrix G_l[H*W, K2*Ho*Wo] (4 nonzeros per column) and the device graph
# becomes pure dense matmuls - no gather ops, which the neuron compiler
# handles poorly.

_LAYERS = [(1, 16, 3, 31, 33), (16, 32, 3, 29, 31), (32, 16, 5, 25, 29),
           (16, 16, 7, 19, 25), (16, 8, 5, 15, 19), (8, 4, 3, 13, 15)]
_B = 256
_NC = 8


def _build_G(offset, K, H, W, Ho, Wo):
    """[H*W, K2*Ho*Wo] bilinear sample+blend matrix from fixed offsets."""
    K2 = K * K
    off = np.asarray(offset, np.float64)[0].reshape(K2, 2, Ho, Wo)
    ky, kx = np.meshgrid(np.arange(K), np.arange(K), indexing='ij')
    py = np.arange(Ho)[None, :, None] + ky.reshape(-1, 1, 1) + off[:, 0]
    px = np.arange(Wo)[None, None, :] + kx.reshape(-1, 1, 1) + off[:, 1]
    y0 = np.floor(py).astype(np.int64); x0 = np.floor(px).astype(np.int64)
    wy = (py - y0).astype(np.float32); wx = (px - x0).astype(np.float32)
    G = np.zeros((H * W, K2 * Ho * Wo), np.float32)
    m = np.arange(K2 * Ho * Wo)
    for dy, wyt in ((0, 1.0 - wy), (1, wy)):
        for dx, wxt in ((0, 1.0 - wx), (1, wx)):
            yi = y0 + dy; xi = x0 + dx
            valid = (yi >= 0) & (yi < H) & (xi >= 0) & (xi < W)
            idx = np.clip(yi, 0, H - 1) * W + np.clip(xi, 0, W - 1)
            wt = (wyt * wxt * valid).reshape(-1).astype(np.float32)
            np.add.at(G, (idx.reshape(-1), m), wt)
    return G


def _forward(x, Gs, ws, bs, w7, b7, w8, b8, perm):
    Bn = x.shape[0]
    for (ci, co, K, ho, hi), G, w, b in zip(_LAYERS, Gs, ws, bs):
        K2 = K * K
        s = x.reshape(Bn, ci, hi * hi) @ G            # [B, ci, K2*ho*ho]
        s = s.reshape(Bn, ci, K2, ho * ho)
        out = jnp.einsum('bckp,ock->bop', s, w.reshape(co, ci, K2))
        x = jax.nn.relu(out + b[None, :, None]).reshape(Bn, co, ho, ho)
    x = x.reshape(Bn, 4, 13 * 13)[:, :, perm].reshape(Bn, -1)
    h = jax.nn.relu(x @ w7 + b7)
    return h @ w8 + b8


_cache = {}


def kernel(**inputs):
    x = np.asarray(inputs['x'], np.float32)
    B = x.shape[0]
    n_dev = min(_NC, len(jax.devices()))
    while B % n_dev != 0:
        n_dev //= 2

    Gs = tuple(_build_G(inputs[f'off{i+1}'], K, hi, hi, ho, ho)
               for i, (ci, co, K, ho, hi) in enumerate(_LAYERS))
    ws = tuple(np.asarray(inputs[f'w{i+1}'], np.float32) for i in range(6))
    bs = tuple(np.asarray(inputs[f'b{i+1}'], np.float32) for i in range(6))
    rest = (np.asarray(inputs['w7'], np.float32), np.asarray(inputs['b7'], np.float32),
            np.asarray(inputs['w8'], np.float32), np.asarray(inputs['b8'], np.float32),
            np.asarray(inputs['perm']))

    if 'fn' not in _cache:
        _cache['fn'] = jax.pmap(
            _forward,
            in_axes=(0,) + (None,) * 8,
            devices=jax.devices()[:n_dev],
        )
    xs = x.reshape(n_dev, B // n_dev, *x.shape[1:])
    out = _cache['fn'](xs, Gs, ws, bs, *rest)
    out = np.asarray(out)
    return out.reshape(B, out.shape[-1]).astype(np.float32)
